# revision 27
# baseline (speedup 1.0000x reference)
"""CenterLoss Trainium2 kernel (8 NeuronCores, data-parallel over batch).

Math: the reference builds the full [N, C] masked distance matrix, but only
the labeled entry of each row survives the mask, so

    loss = ( sum_i ||x_i - centers[labels_i]||^2  +  N*(C-1)*CLAMP_MIN ) / N

(the second term is the clamp applied to the zeroed-out entries).

"select" strategy (v9): the host sorts each core's 2048 samples by label so
each 128-sample tile's labels span < 128 consecutive centers (band trick
from v5). All inputs are packed into ONE fp8 buffer (3 x 128 cols per
tile: transposed one-hot / centers band / x rows), streamed in 3 DMA
chunks. Per tile, one PE matmul gathers the labeled centers into PSUM
(psum[s,d] = sum_w ohT[w,s]*cb[w,d] = centers[label_s, d]), DVE subtracts
x (tensor_tensor, single PSUM operand), and ACT squares + accumulates per
PSUM bank into a [128, 4] fp32 partial that is DMA'd out and summed on
host across cores (the data-parallel all-reduce). fp8 e4m3 input
quantization costs ~6e-4 relative error (gate is 2e-2); the PE gather and
fp32 PSUM difference are exact given the quantized inputs.

Engine budget per core: 3 input DMAs (768 KB), 16 matmuls (~107 ns each,
PE cold/MID pstate), 4 DVE subtracts, 4 ACT square-accumulates, 1 output
DMA. Measured ~18.5-20.4 us wall (run-to-run HW variance ~1 us); fixed
costs dominate: ~7.2 us NEFF startup barrier + instruction load, ~2.3 us
first-DMA latency (HWDGE gen + DGE delay + completion semaphore), ~4 us
output DMA + end barrier. Rejected alternatives (measured slower):
DoubleRow fp8 matmuls (disables fast-weight-load), SWDGE-prepared output
writeback (prep's acc dependency is not deferred to the trigger for
kv_writeback), DVE tensor_tensor_reduce square (runtime INTERNAL error on
hw), 32-matmul -I accumulate variant (PE-bound at ~3.4 us).

Host prep is layout/metadata only: sort + slice + dtype cast of inputs,
one-hot constants. All arithmetic on x and centers happens on device.
Falls back to the v4 indirect-DMA gather kernel if any tile span exceeds
the band width.
"""

import numpy as np

import concourse.bacc as bacc
import concourse.tile as tile
from concourse import bass, mybir
from concourse.bass_utils import run_bass_kernel_spmd

N, C, D = 16384, 1024, 128
N_CORES = 8
NS = N // N_CORES  # 2048 samples per core
P = 128
T = NS // P  # 16 tiles per core
W = 128  # band width
NB = 4  # psum banks / dma chunks
TPB = T // NB  # tiles per bank
CLAMP_MIN = 1e-12

# packed block layout (columns of the [128, BLK_COLS] fp8 buffer), per tile
# t at base = 384*t:
#     [base       : +128]   ohT for tile t  (w -> s)
#     [base + 128 : +128]   centers band    (w -> d)
#     [base + 256 : +128]   x rows          (s -> d)
# Per tile, one PE matmul gathers the labeled centers into PSUM
# (psum[s, d] = sum_w ohT[w,s]*cb[w,d] = centers[label_s, d]), DVE subtracts
# x (one PSUM operand), and ACT squares + accumulates from SBUF.
TILE_COLS = 3 * P  # 384
BLK_COLS = T * TILE_COLS  # 6144
DMA_CHUNKS = (4, 6, 6)  # tiles per input-DMA chunk
ACT_BANKS = (4, 4, 4, 4)  # tiles per PSUM bank / square-accumulate call
DVE_BANKS: set = set()  # banks whose square-accumulate runs on DVE (TTR), not ACT

USE_FP8 = True
BLK_DT = mybir.dt.float8e4 if USE_FP8 else mybir.dt.bfloat16

_cache = {}


def _blk_np_dtype():
    return mybir.dt.np(BLK_DT)


# -------------------------------------------------------------- v6: select
def build_nc_select():
    nc = bacc.Bacc()
    blk = nc.declare_dram_parameter("blk", [P, BLK_COLS], BLK_DT, isOutput=False)
    nout = len(ACT_BANKS)
    out = nc.declare_dram_parameter("out", [P, nout], mybir.dt.float32, isOutput=True)

    with tile.TileContext(nc) as tc:
        with (
            tc.tile_pool(name="data", bufs=1) as data,
            tc.tile_pool(name="psum", bufs=1, space="PSUM") as psump,
        ):
            sb = data.tile([P, BLK_COLS], BLK_DT)
            acc = data.tile([P, nout], mybir.dt.float32)

            # input DMA chunks, alternating sync/scalar rings
            t0 = 0
            for k, nt in enumerate(DMA_CHUNKS):
                a, b = t0 * TILE_COLS, (t0 + nt) * TILE_COLS
                eng = nc.scalar if k % 2 else nc.sync
                eng.dma_start(out=sb[:, a:b], in_=blk[:, a:b])
                t0 += nt

            sb_t = sb[:, :].rearrange("p (t c) -> p t c", c=TILE_COLS)
            t = 0
            for k, nt in enumerate(ACT_BANKS):
                psum_k = psump.tile([P, nt, P], mybir.dt.float32, tag=f"g{k}")
                for i in range(nt):
                    base = (t + i) * TILE_COLS
                    nc.tensor.matmul(
                        out=psum_k[:, i, :],
                        lhsT=sb[:, base : base + P],
                        rhs=sb[:, base + P : base + 2 * P],
                        start=True, stop=True,
                    )
                # difference and square both live in PSUM: ACT's per-call
                # access init is max over operand spaces, and PSUM (172 cy)
                # is cheaper than SBUF (222 cy) on the scalar engine. Each
                # instruction still reads only one PSUM operand (NCC rule).
                d_ps = psump.tile([P, nt, P], mybir.dt.float32, tag=f"d{k}")
                nc.vector.tensor_tensor(
                    out=d_ps[:, :, :],
                    in0=psum_k[:, :, :],
                    in1=sb_t[:, t : t + nt, 2 * P : 3 * P],
                    op=mybir.AluOpType.subtract,
                )
                nc.scalar.activation(
                    out=d_ps[:, :, :],
                    in_=d_ps[:, :, :],
                    func=mybir.ActivationFunctionType.Square,
                    accum_out=acc[:, k : k + 1],
                )
                t += nt
            nc.sync.dma_start(out=out[:, :], in_=acc[:, :])
    nc.compile()
    return nc


def prep_select_core(x_shard, labels_shard, centers_bf):
    """Host layout prep for one core. Returns in_map or None if a tile span
    exceeds the band width."""
    import ml_dtypes

    dt = _blk_np_dtype()
    order = np.argsort(labels_shard, kind="stable")
    ls = labels_shard[order].astype(np.int64)
    bases = np.minimum(ls[::P][:T], C - W)  # [T]
    rel = ls.reshape(T, P) - bases[:, None]  # [T, 128]
    if rel.min() < 0 or rel.max() >= W:
        return None
    xs = np.ascontiguousarray(x_shard[order]).astype(dt)
    iw = np.arange(W)[:, None]
    blk = np.zeros((P, BLK_COLS), dtype=dt)
    for t in range(T):
        base = t * TILE_COLS
        blk[:, base : base + P] = rel[t][None, :] == iw
        blk[:, base + P : base + 2 * P] = centers_bf[bases[t] : bases[t] + W, :]
        blk[:, base + 2 * P : base + 3 * P] = xs[t * P : (t + 1) * P, :]
    return {"blk": blk}


# ------------------------------------------------- v4: indirect-DMA gather
def build_nc_gather(n_chunk=4, n_xdma=4):
    nc = bacc.Bacc()
    x = nc.declare_dram_parameter("x", [NS, D], mybir.dt.float32, isOutput=False)
    centers = nc.declare_dram_parameter(
        "centers", [C, D], mybir.dt.float32, isOutput=False
    )
    labels = nc.declare_dram_parameter("labels", [P, T], mybir.dt.int32, isOutput=False)
    out = nc.declare_dram_parameter("out", [1, 1], mybir.dt.float32, isOutput=True)

    x_t = x.rearrange("(t p) d -> p t d", p=P)
    tpc = T // n_chunk

    with tile.TileContext(nc) as tc:
        with (
            tc.tile_pool(name="data", bufs=1) as data,
            tc.tile_pool(name="small", bufs=1) as small,
            tc.tile_pool(name="psum", bufs=1, space="PSUM") as psump,
        ):
            x_sb = data.tile([P, T, D], mybir.dt.float32)
            g_sb = data.tile([P, T, D], mybir.dt.float32)
            d_sb = data.tile([P, T, D], mybir.dt.float32)
            i_sb = small.tile([P, T], mybir.dt.int32)
            acc = small.tile([P, n_chunk], mybir.dt.float32)
            ones = small.tile([P, 1], mybir.dt.float32)

            nc.vector.memset(ones[:], 1.0)
            nc.sync.dma_start(out=i_sb[:], in_=labels[:, :])
            tpx = T // n_xdma
            for j in range(n_xdma):
                xs = slice(j * tpx, (j + 1) * tpx)
                nc.sync.dma_start(out=x_sb[:, xs, :], in_=x_t[:, xs, :])
            for t in range(T):
                nc.gpsimd.indirect_dma_start(
                    out=g_sb[:, t, :],
                    out_offset=None,
                    in_=centers[:],
                    in_offset=bass.IndirectOffsetOnAxis(ap=i_sb[:, t : t + 1], axis=0),
                )
            for k in range(n_chunk):
                ts = slice(k * tpc, (k + 1) * tpc)
                nc.vector.tensor_tensor(
                    out=d_sb[:, ts, :],
                    in0=x_sb[:, ts, :],
                    in1=g_sb[:, ts, :],
                    op=mybir.AluOpType.subtract,
                )
                nc.scalar.activation(
                    out=d_sb[:, ts, :],
                    in_=d_sb[:, ts, :],
                    func=mybir.ActivationFunctionType.Square,
                    accum_out=acc[:, k : k + 1],
                )
            psum = psump.tile([1, n_chunk], mybir.dt.float32)
            nc.tensor.matmul(
                out=psum[:, :], lhsT=ones[:], rhs=acc[:], start=True, stop=True
            )
            res = small.tile([1, 1], mybir.dt.float32)
            nc.vector.reduce_sum(
                out=res[:1, :1], in_=psum[:1, :], axis=mybir.AxisListType.X
            )
            nc.sync.dma_start(out=out[:, :], in_=res[:1, :1])
    nc.compile()
    return nc


# ----------------------------------------------------------------- driver
def make_in_maps(x, centers, labels):
    """Returns (in_maps, which) where which is 'select' or 'gather'."""
    x = np.ascontiguousarray(np.asarray(x, dtype=np.float32))
    centers = np.ascontiguousarray(np.asarray(centers, dtype=np.float32))
    labels = np.asarray(labels)
    in_maps = []

    centers_bf = np.ascontiguousarray(centers.astype(_blk_np_dtype()))
    for c in range(N_CORES):
        sl = slice(c * NS, (c + 1) * NS)
        m = prep_select_core(x[sl], labels[sl], centers_bf)
        if m is None:
            break
        in_maps.append(m)
    else:
        return in_maps, "select"
    # fallback: indirect gather kernel
    in_maps = []
    for c in range(N_CORES):
        sl = slice(c * NS, (c + 1) * NS)
        in_maps.append(
            {
                "x": x[sl],
                "centers": centers,
                "labels": np.ascontiguousarray(
                    labels[sl].reshape(T, P).T.astype(np.int32)
                ),
            }
        )
    return in_maps, "gather"


def _get_nc(which):
    if which not in _cache:
        _cache[which] = (
            build_nc_select() if which == "select" else build_nc_gather()
        )
    return _cache[which]


def finalize(results, which="select"):
    total = 0.0
    for c in range(N_CORES):
        o = np.asarray(results[c]["out"], dtype=np.float64)
        total += float(o.sum())
    total += N * (C - 1) * CLAMP_MIN
    return np.float32(total / N)


def kernel(x, centers, labels):
    in_maps, which = make_in_maps(x, centers, labels)
    nc = _get_nc(which)
    res = run_bass_kernel_spmd(nc, in_maps, core_ids=list(range(N_CORES)))
    return finalize(res.results)


# revision 29
# speedup vs baseline: 1.1887x; 1.1887x over previous
"""CenterLoss Trainium2 kernel (8 NeuronCores, data-parallel over batch).

Math: the reference builds the full [N, C] masked distance matrix, but only
the labeled entry of each row survives the mask, so

    loss = ( sum_i ||x_i - centers[labels_i]||^2  +  N*(C-1)*CLAMP_MIN ) / N

(the second term is the clamp applied to the zeroed-out entries).

"select" strategy (v9): the host sorts each core's 2048 samples by label so
each 128-sample tile's labels span < 128 consecutive centers (band trick
from v5). All inputs are packed into ONE fp8 buffer (3 x 128 cols per
tile: transposed one-hot / centers band / x rows), streamed in 3 DMA
chunks. Per tile, one PE matmul gathers the labeled centers into PSUM
(psum[s,d] = sum_w ohT[w,s]*cb[w,d] = centers[label_s, d]), DVE subtracts
x (tensor_tensor, single PSUM operand), and ACT squares + accumulates per
PSUM bank into a [128, 4] fp32 partial that is DMA'd out and summed on
host across cores (the data-parallel all-reduce). fp8 e4m3 input
quantization costs ~6e-4 relative error (gate is 2e-2); the PE gather and
fp32 PSUM difference are exact given the quantized inputs.

Engine budget per core: 3 input DMAs (768 KB), 16 matmuls (~107 ns each,
PE cold/MID pstate), 4 DVE subtracts, 4 ACT square-accumulates, 1 output
DMA. Measured ~18.5-20.4 us wall (run-to-run HW variance ~1 us); fixed
costs dominate: ~7.2 us NEFF startup barrier + instruction load, ~2.3 us
first-DMA latency (HWDGE gen + DGE delay + completion semaphore), ~4 us
output DMA + end barrier. Rejected alternatives (measured slower):
DoubleRow fp8 matmuls (disables fast-weight-load), SWDGE-prepared output
writeback (prep's acc dependency is not deferred to the trigger for
kv_writeback), DVE tensor_tensor_reduce square (runtime INTERNAL error on
hw), 32-matmul -I accumulate variant (PE-bound at ~3.4 us).

Host prep is layout/metadata only: sort + slice + dtype cast of inputs,
one-hot constants. All arithmetic on x and centers happens on device.
Falls back to the v4 indirect-DMA gather kernel if any tile span exceeds
the band width.
"""

import numpy as np

import concourse.bacc as bacc
import concourse.tile as tile
from concourse import bass, mybir
from concourse.bass_utils import run_bass_kernel_spmd

N, C, D = 16384, 1024, 128
N_CORES = 8
NS = N // N_CORES  # 2048 samples per core
P = 128
T = NS // P  # 16 tiles per core
W = 128  # band width
NB = 4  # psum banks / dma chunks
TPB = T // NB  # tiles per bank
CLAMP_MIN = 1e-12

# packed block layout (columns of the [128, BLK_COLS] fp8 buffer), per tile
# t at base = 384*t:
#     [base       : +128]   ohT for tile t  (w -> s)
#     [base + 128 : +128]   centers band    (w -> d)
#     [base + 256 : +128]   x rows          (s -> d)
# Per tile, one PE matmul gathers the labeled centers into PSUM
# (psum[s, d] = sum_w ohT[w,s]*cb[w,d] = centers[label_s, d]), DVE subtracts
# x (one PSUM operand), and ACT squares + accumulates from SBUF.
TILE_COLS = 3 * P  # 384
BLK_COLS = T * TILE_COLS  # 6144
DMA_CHUNKS = (4, 6, 6)  # tiles per input-DMA chunk
ACT_BANKS = (4, 4, 4, 4)  # tiles per PSUM bank / square-accumulate call
DVE_BANKS: set = set()  # banks whose square-accumulate runs on DVE (TTR), not ACT

USE_FP8 = True
BLK_DT = mybir.dt.float8e4 if USE_FP8 else mybir.dt.bfloat16

_cache = {}


def _blk_np_dtype():
    return mybir.dt.np(BLK_DT)


# -------------------------------------------------------------- v6: select
def build_nc_select():
    nc = bacc.Bacc()
    blk = nc.declare_dram_parameter("blk", [P, BLK_COLS], BLK_DT, isOutput=False)
    nout = len(ACT_BANKS)
    out = nc.declare_dram_parameter("out", [P, nout], mybir.dt.float32, isOutput=True)

    with tile.TileContext(nc) as tc:
        with (
            tc.tile_pool(name="data", bufs=1) as data,
            tc.tile_pool(name="psum", bufs=1, space="PSUM") as psump,
        ):
            sb = data.tile([P, BLK_COLS], BLK_DT)
            nbmax = max(ACT_BANKS)
            scr0 = data.tile([P, nbmax, P], mybir.dt.bfloat16)
            scr1 = data.tile([P, nbmax, P], mybir.dt.bfloat16)
            scrs = [(scr0, scr1)[k % 2] for k in range(len(ACT_BANKS))]
            acc = data.tile([P, nout], mybir.dt.float32)

            # input DMA chunks, alternating sync/scalar rings
            t0 = 0
            for k, nt in enumerate(DMA_CHUNKS):
                a, b = t0 * TILE_COLS, (t0 + nt) * TILE_COLS
                eng = nc.scalar if k % 2 else nc.sync
                eng.dma_start(out=sb[:, a:b], in_=blk[:, a:b])
                t0 += nt

            sb_t = sb[:, :].rearrange("p (t c) -> p t c", c=TILE_COLS)
            t = 0
            for k, nt in enumerate(ACT_BANKS):
                psum_k = psump.tile([P, nt, P], mybir.dt.float32, tag=f"g{k}")
                for i in range(nt):
                    base = (t + i) * TILE_COLS
                    nc.tensor.matmul(
                        out=psum_k[:, i, :],
                        lhsT=sb[:, base : base + P],
                        rhs=sb[:, base + P : base + 2 * P],
                        start=True, stop=True,
                    )
                # difference lands in an SBUF scratch (a PSUM-resident
                # difference measured 3-4us slower: DVE/ACT/PE contend on
                # the PSUM ports), square-accumulate in place on ACT.
                d_sb = scrs[k]
                nc.vector.tensor_tensor(
                    out=d_sb[:, :nt, :],
                    in0=psum_k[:, :, :],
                    in1=sb_t[:, t : t + nt, 2 * P : 3 * P],
                    op=mybir.AluOpType.subtract,
                )
                nc.scalar.activation(
                    out=d_sb[:, :nt, :],
                    in_=d_sb[:, :nt, :],
                    func=mybir.ActivationFunctionType.Square,
                    accum_out=acc[:, k : k + 1],
                )
                t += nt
            nc.sync.dma_start(out=out[:, :], in_=acc[:, :])
    nc.compile()
    return nc


def prep_select_core(x_shard, labels_shard, centers_bf):
    """Host layout prep for one core. Returns in_map or None if a tile span
    exceeds the band width."""
    import ml_dtypes

    dt = _blk_np_dtype()
    order = np.argsort(labels_shard, kind="stable")
    ls = labels_shard[order].astype(np.int64)
    bases = np.minimum(ls[::P][:T], C - W)  # [T]
    rel = ls.reshape(T, P) - bases[:, None]  # [T, 128]
    if rel.min() < 0 or rel.max() >= W:
        return None
    xs = np.ascontiguousarray(x_shard[order]).astype(dt)
    iw = np.arange(W)[:, None]
    blk = np.zeros((P, BLK_COLS), dtype=dt)
    for t in range(T):
        base = t * TILE_COLS
        blk[:, base : base + P] = rel[t][None, :] == iw
        blk[:, base + P : base + 2 * P] = centers_bf[bases[t] : bases[t] + W, :]
        blk[:, base + 2 * P : base + 3 * P] = xs[t * P : (t + 1) * P, :]
    return {"blk": blk}


# ------------------------------------------------- v4: indirect-DMA gather
def build_nc_gather(n_chunk=4, n_xdma=4):
    nc = bacc.Bacc()
    x = nc.declare_dram_parameter("x", [NS, D], mybir.dt.float32, isOutput=False)
    centers = nc.declare_dram_parameter(
        "centers", [C, D], mybir.dt.float32, isOutput=False
    )
    labels = nc.declare_dram_parameter("labels", [P, T], mybir.dt.int32, isOutput=False)
    out = nc.declare_dram_parameter("out", [1, 1], mybir.dt.float32, isOutput=True)

    x_t = x.rearrange("(t p) d -> p t d", p=P)
    tpc = T // n_chunk

    with tile.TileContext(nc) as tc:
        with (
            tc.tile_pool(name="data", bufs=1) as data,
            tc.tile_pool(name="small", bufs=1) as small,
            tc.tile_pool(name="psum", bufs=1, space="PSUM") as psump,
        ):
            x_sb = data.tile([P, T, D], mybir.dt.float32)
            g_sb = data.tile([P, T, D], mybir.dt.float32)
            d_sb = data.tile([P, T, D], mybir.dt.float32)
            i_sb = small.tile([P, T], mybir.dt.int32)
            acc = small.tile([P, n_chunk], mybir.dt.float32)
            ones = small.tile([P, 1], mybir.dt.float32)

            nc.vector.memset(ones[:], 1.0)
            nc.sync.dma_start(out=i_sb[:], in_=labels[:, :])
            tpx = T // n_xdma
            for j in range(n_xdma):
                xs = slice(j * tpx, (j + 1) * tpx)
                nc.sync.dma_start(out=x_sb[:, xs, :], in_=x_t[:, xs, :])
            for t in range(T):
                nc.gpsimd.indirect_dma_start(
                    out=g_sb[:, t, :],
                    out_offset=None,
                    in_=centers[:],
                    in_offset=bass.IndirectOffsetOnAxis(ap=i_sb[:, t : t + 1], axis=0),
                )
            for k in range(n_chunk):
                ts = slice(k * tpc, (k + 1) * tpc)
                nc.vector.tensor_tensor(
                    out=d_sb[:, ts, :],
                    in0=x_sb[:, ts, :],
                    in1=g_sb[:, ts, :],
                    op=mybir.AluOpType.subtract,
                )
                nc.scalar.activation(
                    out=d_sb[:, ts, :],
                    in_=d_sb[:, ts, :],
                    func=mybir.ActivationFunctionType.Square,
                    accum_out=acc[:, k : k + 1],
                )
            psum = psump.tile([1, n_chunk], mybir.dt.float32)
            nc.tensor.matmul(
                out=psum[:, :], lhsT=ones[:], rhs=acc[:], start=True, stop=True
            )
            res = small.tile([1, 1], mybir.dt.float32)
            nc.vector.reduce_sum(
                out=res[:1, :1], in_=psum[:1, :], axis=mybir.AxisListType.X
            )
            nc.sync.dma_start(out=out[:, :], in_=res[:1, :1])
    nc.compile()
    return nc


# ----------------------------------------------------------------- driver
def make_in_maps(x, centers, labels):
    """Returns (in_maps, which) where which is 'select' or 'gather'."""
    x = np.ascontiguousarray(np.asarray(x, dtype=np.float32))
    centers = np.ascontiguousarray(np.asarray(centers, dtype=np.float32))
    labels = np.asarray(labels)
    in_maps = []

    centers_bf = np.ascontiguousarray(centers.astype(_blk_np_dtype()))
    for c in range(N_CORES):
        sl = slice(c * NS, (c + 1) * NS)
        m = prep_select_core(x[sl], labels[sl], centers_bf)
        if m is None:
            break
        in_maps.append(m)
    else:
        return in_maps, "select"
    # fallback: indirect gather kernel
    in_maps = []
    for c in range(N_CORES):
        sl = slice(c * NS, (c + 1) * NS)
        in_maps.append(
            {
                "x": x[sl],
                "centers": centers,
                "labels": np.ascontiguousarray(
                    labels[sl].reshape(T, P).T.astype(np.int32)
                ),
            }
        )
    return in_maps, "gather"


def _get_nc(which):
    if which not in _cache:
        _cache[which] = (
            build_nc_select() if which == "select" else build_nc_gather()
        )
    return _cache[which]


def finalize(results, which="select"):
    total = 0.0
    for c in range(N_CORES):
        o = np.asarray(results[c]["out"], dtype=np.float64)
        total += float(o.sum())
    total += N * (C - 1) * CLAMP_MIN
    return np.float32(total / N)


def kernel(x, centers, labels):
    in_maps, which = make_in_maps(x, centers, labels)
    nc = _get_nc(which)
    res = run_bass_kernel_spmd(nc, in_maps, core_ids=list(range(N_CORES)))
    return finalize(res.results)
